# revision 44
# baseline (speedup 1.0000x reference)
"""DeepSAT GNN message-passing kernel for 8 Trainium2 NeuronCores.

Algorithm notes (validated numerically against the reference):
  - Every node is updated exactly once, at step l = forward_level (levels
    1..19; level-0 nodes keep h0 forever). At update time the node's own
    hidden state is still h0, so the GRU "hidden side" gates are constant
    vectors computable on the host.
  - msg_i = W @ (S_i + n0_i*h0) + deg_i*b, where S_i sums h over "active"
    in-edges (source level in [1, level_i)), n0_i counts inactive in-edges
    and deg_i all in-edges. With u = W^-1 b this folds to
    msg_i = W @ S'_i,  S'_i = S_i + n0_i*h0 + deg_i*u, so the per-gate
    input is  gi_g = (wih_g @ W) @ S'_i + bih_g  -- one fused matmul.
  - Nodes are stored level-sorted ("rank" order): per-level writes are
    contiguous, per-level ownership is an even 8-way split, and the
    AllGather of each level's new h lands in place.

Perf design (v2):
  - All matmuls/h-storage in bf16 (PE runs 1 cycle/row vs 4 for fp32);
    PSUM accumulation stays fp32.
  - Edge gathers use gpsimd.dma_gather (one SWDGE instruction per up to
    ~2-3k edges) instead of one indirect DMA per 128 edges: SWDGE fixed
    overhead is ~1us per instruction and was the previous bottleneck.
    dma_gather indices are int16, so "old" gathers (sources anywhere
    below the current level) are windowed into 32768-row slabs of
    h_store; "fresh" gathers (sources in level l-1 only) use the l-1
    slab directly.
  - Edges are sorted by destination slot so each 128-edge chunk spans
    few 128-slot blocks (~85% fill vs 44% before): fewer descriptors
    and fewer one-hot segment-sum matmuls.

Device schedule per level l (SPMD on 8 cores):
  dma_gather h[src] for this level's "fresh" edges (src level == l-1),
  segment-sum via one-hot matmuls into PSUM (seeded with the n0/deg
  terms), fused GRU, PE-transpose, DMA to the AllGather input, AllGather
  into the replicated h_store, then (overlapping the collective) the MLP
  head for this level plus the next level's "old" gathers/chunks, whose
  windows only read rows below this level's slab.
"""

import sys
import numpy as np

sys.path.insert(0, "/opt/trn_rl_repo")

P = 128
D = 128
NC = 8
GW = 512  # psum group width (one bank of fp32)
W32 = 32768  # dma_gather int16 index window

_COMPILED = {}


def _bf16():
    import ml_dtypes
    return ml_dtypes.bfloat16


# ---------------------------------------------------------------------------
# Host-side preprocessing
# ---------------------------------------------------------------------------

def _wrap_idx(vals):
    """int16 idx layout: j at [j%16, j//16], replicated to 8x16 partitions."""
    n = len(vals)
    cols = (n + 15) // 16
    t = np.zeros((P, cols), np.int16)
    for k in range(8):
        t[16 * k + (np.arange(n) % 16), np.arange(n) // 16] = vals
    return t


def _preprocess(forward_level, edge_index, num_levels):
    fl = np.asarray(forward_level).astype(np.int64)
    ei = np.asarray(edge_index).astype(np.int64)
    src, dst = ei[0], ei[1]
    N = fl.shape[0]
    NL = num_levels

    # --- rank space: nodes sorted by level, each level padded to NC*P ---
    n_l = np.bincount(fl, minlength=NL).astype(np.int64)
    pad_l = ((n_l + NC * P - 1) // (NC * P)) * (NC * P)
    pad_l = np.maximum(pad_l, NC * P)
    L_off = np.zeros(NL + 1, np.int64)
    L_off[1:] = np.cumsum(pad_l)
    Vc = (pad_l // NC).astype(np.int64)          # per-core nodes per level
    Voff = np.zeros(NL + 1, np.int64)
    Voff[1:] = np.cumsum(Vc)                     # per-core rank-space offsets
    nblk = (Vc // P).astype(np.int64)

    order = np.argsort(fl, kind="stable")
    starts_real = np.zeros(NL + 1, np.int64)
    starts_real[1:] = np.cumsum(n_l)
    pos_within = np.arange(N, dtype=np.int64) - starts_real[fl[order]]
    rank = np.empty(N, np.int64)
    rank[order] = L_off[fl[order]] + pos_within

    node_of_rank = np.full(L_off[NL], -1, np.int64)
    node_of_rank[rank] = np.arange(N, dtype=np.int64)

    # --- per-node degree stats, indexed by per-core rank space ---
    lv_s, lv_d = fl[src], fl[dst]
    act = (lv_s >= 1) & (lv_s < lv_d)
    deg = np.bincount(dst, minlength=N).astype(np.float64)
    n0 = np.bincount(dst[~act], minlength=N).astype(np.float64)

    sumVc = int(Voff[NL])
    n0row = np.zeros((NC, sumVc), np.float32)
    degrow = np.zeros((NC, sumVc), np.float32)
    for c in range(NC):
        grs = []
        for l in range(NL):
            grs.append(L_off[l] + c * Vc[l] + np.arange(Vc[l]))
        gr = np.concatenate(grs)
        nd = node_of_rank[gr]
        m = nd >= 0
        n0row[c, m] = n0[nd[m]]
        degrow[c, m] = deg[nd[m]]

    # --- active edge table ---
    er = np.where(act)[0]
    e_lvl = lv_d[er]
    e_srcrank = rank[src[er]].astype(np.int64)
    e_dstrank = rank[dst[er]].astype(np.int64)
    e_local = e_dstrank - L_off[e_lvl]
    e_core = e_local // Vc[e_lvl]
    e_wl = e_local % Vc[e_lvl]          # slot within core's level span
    e_fresh = lv_s[er] == (e_lvl - 1)

    # Per-iteration gather buckets; edges sorted by (dst level, dst slot) so
    # chunks span few 128-slot blocks. All counts are the max over cores so
    # the SPMD program is identical everywhere.
    #
    #   iter t "fm":    src level == t-1, dst in {t, t+1}. One window (the
    #                   l-1 slab); fires right after AllGather t-1 -- the
    #                   only gather on the critical path.
    #   iter t "stale": src level <= t-2, dst == t+1 (32k-row windows).
    #                   Emitted after collective_t: all its AllGather deps
    #                   completed long before, so descriptor generation and
    #                   the DMA overlap AllGather t entirely.
    idx_cols = []                # list of [NC, n] int64 idx blocks
    icol = 0
    rank_cols = [[] for _ in range(NC)]  # per pair: [128] f32 block-rel slots
    e_slot = e_wl                # dst slot within the core's level span

    def build_bucket(buckets):
        """buckets: list of (base, rows, sel_mask, padidx). Returns bucket
        dict with gather instrs, chunk count and pairs (lvl, ch, grp, big,
        pcol), or None if empty."""
        nonlocal icol
        out = {"instrs": [], "pairs": [], "nchunks": 0}
        nch_tot = 0
        for base, rows, sel, padidx in buckets:
            percore = []
            for c in range(NC):
                es = np.where(sel & (e_core == c))[0]
                es = es[np.lexsort((e_slot[es], e_lvl[es]))]
                percore.append(es)
            nmax = max(len(x) for x in percore)
            if nmax == 0:
                continue
            n = ((nmax + P - 1) // P) * P
            nch = n // P
            iv = np.zeros((NC, n), np.int64)
            for c in range(NC):
                es = percore[c]
                iv[c, :len(es)] = e_srcrank[es] - base
                iv[c, len(es):] = padidx
            assert iv.min() >= 0 and iv.max() < min(rows, W32)
            out["instrs"].append({"base": base, "rows": rows, "n": n,
                                  "icol": icol, "chunk0": nch_tot})
            idx_cols.append(iv)
            icol += n // 16
            for ch in range(nch):
                sl = slice(ch * P, (ch + 1) * P)
                lvl_blocks = set()
                for c in range(NC):
                    es = percore[c][sl]
                    lvl_blocks.update(
                        zip(e_lvl[es].tolist(), (e_slot[es] // P).tolist()))
                for (lv, b) in sorted(lvl_blocks):
                    pcol = len(rank_cols[0])
                    for c in range(NC):
                        es = percore[c][sl]
                        rv = np.full(P, -1.0, np.float32)
                        m = (e_lvl[es] == lv) & ((e_slot[es] // P) == b)
                        rv[:len(es)][m] = (e_slot[es][m] - b * P).astype(
                            np.float32)
                        rank_cols[c].append(rv)
                    out["pairs"].append(
                        (int(lv), nch_tot + ch, int(b) // 4, int(b) % 4, pcol))
            nch_tot += nch
        out["nchunks"] = nch_tot
        return out if nch_tot else None

    iters = [None] * NL
    for t in range(1, NL):
        fm = None
        if t >= 2:
            base = int(L_off[t - 1])
            assert pad_l[t - 1] <= W32, "slab exceeds int16 idx range"
            sel = (lv_s[er] == t - 1) & ((e_lvl == t) | (e_lvl == t + 1))
            fm = build_bucket([(base, int(pad_l[t - 1]), sel, 0)])
        stale = None
        if t + 1 < NL and t >= 3:
            sel0 = (e_lvl == t + 1) & (lv_s[er] <= t - 2) & (lv_s[er] >= 1)
            max_row = int(L_off[t - 1])
            nw = (max_row + W32 - 1) // W32
            bks = []
            for w in range(nw):
                wsel = sel0 & (e_srcrank >= w * W32) & (e_srcrank < (w + 1) * W32)
                rows = min(W32, int(L_off[NL]) - w * W32)
                padidx = int(L_off[1]) if w == 0 else 0
                bks.append((w * W32, rows, wsel, padidx))
            stale = build_bucket(bks)
        iters[t] = {"fm": fm, "stale": stale}

    # stop-flag bookkeeping: matmuls into S_ps[l] run in emission order
    # seeds_l -> stale pairs (iter l-1) -> mid pairs (fm of iter l-1,
    # lvl==l) -> fresh pairs (fm of iter l, lvl==l)
    levels = []
    for l in range(NL):
        info = {"ngrp": (int(Vc[l]) + GW - 1) // GW, "last": {}}
        if l >= 1:
            seqs = []
            if l >= 2 and iters[l - 1] and iters[l - 1]["stale"]:
                seqs.append(iters[l - 1]["stale"]["pairs"])
            if l >= 2 and iters[l - 1] and iters[l - 1]["fm"]:
                seqs.append(iters[l - 1]["fm"]["pairs"])
            if iters[l] and iters[l]["fm"]:
                seqs.append(iters[l]["fm"]["pairs"])
            for pairs in seqs:
                for (lv, ch, grp, big, pcol) in pairs:
                    if lv == l:
                        info["last"][grp] = pcol
        levels.append(info)

    ICOLS = max(icol, 1)
    idxs = np.zeros((NC, P, ICOLS), np.int16)
    bi = 0
    for t in range(1, NL):
        for bucket in (iters[t]["fm"], iters[t]["stale"]) if iters[t] else ():
            if bucket is None:
                continue
            for ins in bucket["instrs"]:
                iv = idx_cols[bi]
                bi += 1
                cols = iv.shape[1] // 16
                for c in range(NC):
                    idxs[c][:, ins["icol"]:ins["icol"] + cols] = _wrap_idx(iv[c])
    assert bi == len(idx_cols)
    # host-built one-hot matrices, DMA'd to SBUF instead of generated on
    # the vector engine (is_equal over ~20M elems was pacing the tail)
    NPAIR = max(len(rank_cols[0]), 1)
    bf16 = _bf16()
    onehots = np.zeros((NC, P, NPAIR * P), bf16)
    iota = np.arange(P, dtype=np.float32)
    for c in range(NC):
        if rank_cols[c]:
            rk = np.stack(rank_cols[c], axis=1)      # [128, npair]
            oh = (rk[:, :, None] == iota[None, None, :])  # [128, npair, 128]
            onehots[c, :, :oh.shape[1] * P] = oh.reshape(P, -1).astype(bf16)

    return {
        "N": N, "NL": NL, "n_l": n_l, "pad": pad_l, "L_off": L_off,
        "Vc": Vc, "Voff": Voff, "nblk": nblk, "sumVc": sumVc,
        "ICOLS": ICOLS, "NPAIR": NPAIR,
        "levels": levels, "iters": iters, "idxs": idxs, "onehots": onehots,
        "n0row": n0row, "degrow": degrow, "node_of_rank": node_of_rank,
    }


def _prep_weights(inp):
    f64 = np.float64
    W = inp["aggr_w"].astype(f64)
    b = inp["aggr_b"].astype(f64)
    h0 = (inp["emd_w"][:, 0] + inp["emd_b"]).astype(f64)
    wih = inp["gru_wih"].astype(f64)
    whh = inp["gru_whh"].astype(f64)
    bih = inp["gru_bih"].astype(f64)
    bhh = inp["gru_bhh"].astype(f64)
    u = np.linalg.solve(W, b)
    assert np.abs(W @ u - b).max() < 1e-6
    ghc = whh @ h0 + bhh
    hr_c, hz_c, hn_c = ghc[:D], ghc[D:2 * D], ghc[2 * D:]
    bih_r, bih_z, bih_n = bih[:D], bih[D:2 * D], bih[2 * D:]
    WgT = [(wih[g * D:(g + 1) * D] @ W).T for g in range(3)]

    W1 = inp["w1"].astype(f64)  # [256, 128]
    W2 = inp["w2"].astype(f64)  # [256, 256]
    w3 = inp["w3"].astype(f64)  # [1, 256]
    assert W1.shape[0] == 256

    bf16 = _bf16()
    blocks = [
        WgT[0], WgT[1], WgT[2], np.diag(hn_c),
        W1[0:128, :].T, W1[128:256, :].T,
        W2[0:128, 0:128].T, W2[0:128, 128:256].T,
        W2[128:256, 0:128].T, W2[128:256, 128:256].T,
        np.eye(128), np.tile(np.arange(128, dtype=f64)[None, :], (128, 1)),
        np.concatenate([w3[0, 0:128, None], w3[0, 128:256, None],
                        np.zeros((128, 126))], axis=1),
    ]
    wmat = np.concatenate(blocks, axis=1).astype(bf16)  # [128, 13*128] bf16

    vcols = np.stack([
        h0,                      # 0: h0 column
        bih_r + hr_c,            # 1: sigmoid bias for r
        -(bih_z + hz_c),         # 2: sigmoid bias for z' (scale = -1)
        bih_n,                   # 3: tanh bias for n
        inp["b1"].astype(f64)[0:128],    # 4
        inp["b1"].astype(f64)[128:256],  # 5
        inp["b2"].astype(f64)[0:128],    # 6
        inp["b2"].astype(f64)[128:256],  # 7
        np.full(128, inp["b3"].astype(f64)[0]),  # 8: b3 (row 0 used)
    ], axis=1).astype(np.float32)  # [128, 9] fp32 (activation biases + h0)

    vrow = np.zeros((1, 256), np.float32)
    vrow[0, :128] = h0.astype(np.float32)
    vrow[0, 128:] = u.astype(np.float32)
    return wmat, vcols, vrow.astype(bf16)


# ---------------------------------------------------------------------------
# Bass program
# ---------------------------------------------------------------------------

WM = {name: i for i, name in enumerate(
    ["WgT_r", "WgT_z", "WgT_n", "diag_hn", "W1Ta", "W1Tb",
     "W2_k0m0", "W2_k1m0", "W2_k0m1", "W2_k1m1", "ident", "iota", "w3c"])}
VC = {name: i for i, name in enumerate(
    ["h0", "bias_r", "nbias_z", "bias_n", "b1a", "b1b", "b2a", "b2b", "b3"])}




def _build(sched, reps=1):
    import concourse.bacc as bacc
    import concourse.tile as tile
    from concourse import bass, mybir, library_config

    f32 = mybir.dt.float32
    bf = mybir.dt.bfloat16
    i16 = mybir.dt.int16
    AF = mybir.ActivationFunctionType
    OP = mybir.AluOpType
    NL = sched["NL"]
    L_off = sched["L_off"]
    Vc = sched["Vc"]
    Voff = sched["Voff"]
    pad = sched["pad"]
    ICOLS = sched["ICOLS"]
    NPAIR = sched["NPAIR"]
    sumVc = sched["sumVc"]
    NpadTot = int(L_off[NL])
    RG = [list(range(NC))]

    nc = bacc.Bacc("TRN2", target_bir_lowering=False, debug=False,
                   enable_asserts=False, num_devices=NC)

    wmat_d = nc.dram_tensor("wmat", [P, P * len(WM)], bf, kind="ExternalInput")
    vcols_d = nc.dram_tensor("vcols", [P, len(VC)], f32, kind="ExternalInput")
    vrow_d = nc.dram_tensor("vrow", [1, 256], bf, kind="ExternalInput")
    n0_d = nc.dram_tensor("n0row", [1, sumVc], bf, kind="ExternalInput")
    deg_d = nc.dram_tensor("degrow", [1, sumVc], bf, kind="ExternalInput")
    idx_d = nc.dram_tensor("idxs", [P, ICOLS], i16, kind="ExternalInput")
    oh_d = nc.dram_tensor("onehots", [P, NPAIR * P], bf, kind="ExternalInput")
    pred_d = nc.dram_tensor("pred", [sumVc], f32, kind="ExternalOutput")
    h_store = nc.dram_tensor("h_store", [NpadTot, D], bf, kind="Internal",
                             addr_space="Shared")
    ag_in = [nc.dram_tensor(f"ag_in{i}", [int(Vc.max()), D], bf, kind="Internal")
             for i in range(2)]

    with tile.TileContext(nc) as tc:
        nc.gpsimd.load_library(library_config.mlp)
        cpool = tc.alloc_tile_pool(name="const", bufs=1)
        spool = tc.alloc_tile_pool(name="sbuf", bufs=2)
        gpool = tc.alloc_tile_pool(name="gath", bufs=2)
        hpool = tc.alloc_tile_pool(name="hnew", bufs=6)
        ppool = tc.alloc_tile_pool(name="psS", bufs=3, space="PSUM")
        qpool = tc.alloc_tile_pool(name="psG", bufs=3, space="PSUM")
        tpool = tc.alloc_tile_pool(name="psT", bufs=1, space="PSUM")
        rpool = tc.alloc_tile_pool(name="psP", bufs=1, space="PSUM")

        # ---- load constants ----
        wm = cpool.tile([P, P * len(WM)], bf, tag="wm")
        nc.sync.dma_start(out=wm[:], in_=wmat_d[:])
        vc = cpool.tile([P, len(VC)], f32, tag="vc")
        nc.sync.dma_start(out=vc[:], in_=vcols_d[:])
        vr = cpool.tile([1, 256], bf, tag="vr")
        nc.sync.dma_start(out=vr[:], in_=vrow_d[:])
        idxs = cpool.tile([P, ICOLS], i16, tag="idxs")
        nc.sync.dma_start(out=idxs[:], in_=idx_d[:])

        def wmb(name):
            return wm[:, WM[name] * P:(WM[name] + 1) * P]

        def vcc(name):
            return vc[:, VC[name]:VC[name] + 1]

        h0b = cpool.tile([P, GW], bf, tag="h0b")  # h0 broadcast along free
        nc.vector.tensor_copy(out=h0b[:], in_=vcc("h0").to_broadcast([P, GW]))

        # ---- per-level state ----
        S_ps = [None] * NL         # list of psum tiles per level (by grp)
        last_ag = [None]           # most recent AllGather instruction

        def grp_widths(l):
            ws = []
            v = int(Vc[l])
            while v > 0:
                ws.append(min(GW, v))
                v -= GW
            return ws

        def emit_gathers(bucket, tag, pin):
            """dma_gather(s) for one bucket; pin = AllGather whose output
            the windows may read (DRAM regions aren't shadow-tracked)."""
            if bucket is None:
                return None
            hg = gpool.tile([P, bucket["nchunks"] * D], bf, tag=tag)
            for ins in bucket["instrs"]:
                n = ins["n"]
                c0 = ins["chunk0"]
                gi = nc.gpsimd.dma_gather(
                    out_ap=hg[:, c0 * D:(c0 + n // P) * D].rearrange(
                        "p (k d) -> p k d", d=D),
                    in_ap=h_store[ins["base"]:ins["base"] + ins["rows"], :],
                    idxs_ap=idxs[:, ins["icol"]:ins["icol"] + n // 16],
                    num_idxs=n,
                    num_idxs_reg=n,
                    elem_size=D,
                )
                if pin is not None:
                    tile.add_dep_helper(gi.ins, pin.ins, sync=True,
                                        reason="gather reads AllGather output")
            return hg

        def emit_onehots(bucket, tag):
            """DMA the bucket's host-built one-hot block into SBUF."""
            if bucket is None or not bucket["pairs"]:
                return None, None
            k = len(bucket["pairs"])
            p0 = bucket["pairs"][0][4]
            oh = spool.tile([P, k * P], bf, tag=tag)
            nc.scalar.dma_start(out=oh[:], in_=oh_d[:, p0 * P:(p0 + k) * P])
            return oh, p0

        def emit_seeds(l):
            """allocate S psums for level l and seed with n0*h0 + deg*u."""
            tiles = []
            info = sched["levels"][l]
            v = int(Vc[l])
            off = int(Voff[l])
            n0r = spool.tile([1, int(Vc.max())], bf, tag="n0r")
            nc.sync.dma_start(out=n0r[0:1, :v], in_=n0_d[0:1, off:off + v])
            degr = spool.tile([1, int(Vc.max())], bf, tag="degr")
            nc.sync.dma_start(out=degr[0:1, :v], in_=deg_d[0:1, off:off + v])
            for g, w in enumerate(grp_widths(l)):
                sp = ppool.tile([P, GW], f32, tag="S", space="PSUM")
                nc.tensor.matmul(
                    out=sp[:, :w], lhsT=vr[0:1, 0:128],
                    rhs=n0r[0:1, g * GW:g * GW + w],
                    start=True, stop=False, skip_group_check=True)
                is_last = info["last"].get(g) is None
                nc.tensor.matmul(
                    out=sp[:, :w], lhsT=vr[0:1, 128:256],
                    rhs=degr[0:1, g * GW:g * GW + w],
                    start=False, stop=is_last, skip_group_check=True)
                tiles.append(sp)
            S_ps[l] = tiles

        def emit_chunks(bucket, lvl, hg, oh, p0):
            """segment-sum matmuls for the bucket's pairs targeting S_ps[lvl]."""
            if bucket is None:
                return
            info = sched["levels"][lvl]
            for (lv, ch, grp, big, pcol) in bucket["pairs"]:
                if lv != lvl:
                    continue
                is_last = info["last"].get(grp) == pcol
                nc.tensor.matmul(
                    out=S_ps[lvl][grp][:, big * P:(big + 1) * P],
                    lhsT=hg[:, ch * D:(ch + 1) * D],
                    rhs=oh[:, (pcol - p0) * P:(pcol - p0 + 1) * P],
                    start=False, stop=is_last, skip_group_check=True)

        def emit_mlp(l, g, w, rhs_sb, bcast=False):
            """MLP head for one 512-group; writes pred rows."""
            z1s = []
            for half in ("a", "b"):
                zp = qpool.tile([P, GW], f32, tag="G", space="PSUM")
                nc.tensor.matmul(out=zp[:, :w], lhsT=wmb("W1T" + half),
                                 rhs=rhs_sb[:, :w], start=True, stop=True)
                zs = spool.tile([P, GW], bf, tag="z1" + half)
                nc.scalar.activation(out=zs[:, :w], in_=zp[:, :w], func=AF.Relu,
                                     bias=vcc("b1" + half))
                z1s.append(zs)
            z2s = []
            for mi, mh in enumerate(("m0", "m1")):
                zp = qpool.tile([P, GW], f32, tag="G", space="PSUM")
                nc.tensor.matmul(out=zp[:, :w], lhsT=wmb("W2_k0" + mh),
                                 rhs=z1s[0][:, :w], start=True, stop=False)
                nc.tensor.matmul(out=zp[:, :w], lhsT=wmb("W2_k1" + mh),
                                 rhs=z1s[1][:, :w], start=False, stop=True)
                zs = spool.tile([P, GW], bf, tag="z2" + mh)
                nc.scalar.activation(out=zs[:, :w], in_=zp[:, :w], func=AF.Relu,
                                     bias=vcc("b2" + ("a" if mi == 0 else "b")))
                z2s.append(zs)
            pp = rpool.tile([1, GW], f32, tag="pred", space="PSUM")
            nc.tensor.matmul(out=pp[:, :w], lhsT=wmb("w3c")[:, 0:1],
                             rhs=z2s[0][:, :w], start=True, stop=False)
            nc.tensor.matmul(out=pp[:, :w], lhsT=wmb("w3c")[:, 1:2],
                             rhs=z2s[1][:, :w], start=False, stop=True)
            ps = spool.tile([1, GW], f32, tag="psb")
            nc.scalar.activation(out=ps[:, :w], in_=pp[:, :w], func=AF.Identity,
                                 bias=vc[0:1, VC["b3"]:VC["b3"] + 1])
            if bcast:
                pbc = spool.tile([1, GW], f32, tag="pbc")
                nc.vector.tensor_copy(out=pbc[0:1, :],
                                      in_=ps[0:1, 0:1].to_broadcast([1, GW]))
                for gg, ww in enumerate(grp_widths(l)):
                    off = int(Voff[l]) + gg * GW
                    nc.sync.dma_start(out=pred_d[off:off + ww],
                                      in_=pbc[0:1, :ww])
            else:
                off = int(Voff[l]) + g * GW
                nc.sync.dma_start(out=pred_d[off:off + w], in_=ps[0:1, :w])

        # reps>1 repeats the whole computation for wall-clock timing: the
        # computation is idempotent (h_store/pred rewritten with same values)
        for _rep in range(reps):
          # ================= level 0: one column, broadcast ==============
          # every level-0 node keeps h = h0, so pred is a single scalar
          emit_mlp(0, 0, 1, h0b, bcast=True)

          # seeds + (no chunks possible) for level 1
          emit_seeds(1)
          OH = {}  # (t, kind) -> (oh, p0); prefetched one iteration ahead

          def prefetch_oh(t):
              if t < NL and sched["iters"][t]:
                  OH[(t, "fm")] = emit_onehots(sched["iters"][t]["fm"], "oh_fm")
                  OH[(t, "st")] = emit_onehots(sched["iters"][t]["stale"], "oh_st")

          prefetch_oh(1)

          # ================= levels 1..NL-1 =================
          for l in range(1, NL):
            widths = grp_widths(l)
            it = sched["iters"][l]
            fm = it["fm"] if it else None
            stale = it["stale"] if it else None

            # fm gather: the only one on the critical path (needs AG l-1);
            # reads the l-1 slab for fresh_l + mid_{l+1} edges
            hg_fm = emit_gathers(fm, "hg_fm", last_ag[0])
            oh_fm, p0_fm = OH.get((l, "fm"), (None, None))
            if hg_fm is not None:
                emit_chunks(fm, l, hg_fm, oh_fm, p0_fm)

            # GRU per group
            hnew = []
            for g, w in enumerate(widths):
                ssb = spool.tile([P, GW], bf, tag="Ssb")
                nc.vector.tensor_copy(out=ssb[:, :w], in_=S_ps[l][g][:, :w])

                gr = qpool.tile([P, GW], f32, tag="G", space="PSUM")
                nc.tensor.matmul(out=gr[:, :w], lhsT=wmb("WgT_r"),
                                 rhs=ssb[:, :w], start=True, stop=True)
                gz = qpool.tile([P, GW], f32, tag="G", space="PSUM")
                nc.tensor.matmul(out=gz[:, :w], lhsT=wmb("WgT_z"),
                                 rhs=ssb[:, :w], start=True, stop=True)
                gn = qpool.tile([P, GW], f32, tag="G", space="PSUM")
                nc.tensor.matmul(out=gn[:, :w], lhsT=wmb("WgT_n"),
                                 rhs=ssb[:, :w], start=True, stop=False)

                rsb = spool.tile([P, GW], bf, tag="rsb")
                nc.scalar.activation(out=rsb[:, :w], in_=gr[:, :w],
                                     func=AF.Sigmoid, bias=vcc("bias_r"))
                zsb = spool.tile([P, GW], bf, tag="zsb")
                nc.scalar.activation(out=zsb[:, :w], in_=gz[:, :w],
                                     func=AF.Sigmoid, bias=vcc("nbias_z"),
                                     scale=-1.0)
                nc.tensor.matmul(out=gn[:, :w], lhsT=wmb("diag_hn"),
                                 rhs=rsb[:, :w], start=False, stop=True)
                nsb = spool.tile([P, GW], bf, tag="nsb")
                nc.scalar.activation(out=nsb[:, :w], in_=gn[:, :w],
                                     func=AF.Tanh, bias=vcc("bias_n"))

                t3 = spool.tile([P, GW], bf, tag="t3")
                nc.vector.tensor_scalar(out=t3[:, :w], in0=nsb[:, :w],
                                        scalar1=vcc("h0"), scalar2=None,
                                        op0=OP.subtract)
                t4 = spool.tile([P, GW], bf, tag="t4")
                nc.vector.tensor_tensor(out=t4[:, :w], in0=t3[:, :w],
                                        in1=zsb[:, :w], op=OP.mult)
                hn = hpool.tile([P, GW], bf, tag="hnew")
                nc.vector.tensor_scalar(out=hn[:, :w], in0=t4[:, :w],
                                        scalar1=vcc("h0"), scalar2=None,
                                        op0=OP.add)
                hnew.append(hn)

            # transpose h_new to node-major, stage straight from PSUM, and
            # AllGather into every core's h_store (skipped for the last
            # level: nothing reads it)
            if l < NL - 1:
                agt = ag_in[l % 2]
                for g, w in enumerate(widths):
                    tp = tpool.tile([P, GW], bf, tag="tp", space="PSUM")
                    nb = w // P
                    for b in range(nb):
                        nc.tensor.transpose(
                            out=tp[:, b * P:(b + 1) * P],
                            in_=hnew[g][:, b * P:(b + 1) * P],
                            identity=wmb("ident"))
                    tps = spool.tile([P, GW], bf, tag="tps")
                    nc.vector.tensor_copy(out=tps[:, :w], in_=tp[:, :w])
                    for b in range(nb):
                        row = g * GW + b * P
                        nc.sync.dma_start(out=agt[row:row + P, :],
                                          in_=tps[:, b * P:(b + 1) * P])
            prev_ag = last_ag[0]
            if l < NL - 1:
                cc = nc.gpsimd.collective_compute(
                    "AllGather", mybir.AluOpType.bypass,
                    replica_groups=RG,
                    ins=[agt[0:int(Vc[l]), :].opt()],
                    outs=[h_store[int(L_off[l]):int(L_off[l]) + int(pad[l]), :].opt()],
                )
                last_ag[0] = cc

            # while AllGather l flies: seeds, stale gathers (their newest
            # source level is l-2, so AG l-1 -- long since waited on -- is
            # their pin), stale + mid chunks for level l+1, this level's
            # MLP head, and the one-hot prefetch for the next iteration
            if l + 1 < NL:
                emit_seeds(l + 1)
                hg_st = emit_gathers(stale, "hg_st", prev_ag)
                oh_st, p0_st = OH.get((l, "st"), (None, None))
                if hg_st is not None:
                    emit_chunks(stale, l + 1, hg_st, oh_st, p0_st)
                if hg_fm is not None:
                    emit_chunks(fm, l + 1, hg_fm, oh_fm, p0_fm)

            for g, w in enumerate(widths):
                emit_mlp(l, g, w, hnew[g])

            prefetch_oh(l + 1)

        for pl in (rpool, tpool, qpool, ppool, hpool, gpool, spool, cpool):
            pl.release()

    nc.compile()
    return nc


# ---------------------------------------------------------------------------
# Entry point
# ---------------------------------------------------------------------------

def _run(inputs, trace=False, reps=1):
    from concourse.bass_utils import run_bass_kernel_spmd

    bf16 = _bf16()
    fl = np.asarray(inputs["forward_level"])
    num_levels = int(fl.max()) + 1
    sched = _preprocess(fl, inputs["edge_index"], num_levels)
    wmat, vcols, vrow = _prep_weights(inputs)

    key = (sched["N"], sched["ICOLS"], sched["NPAIR"], sched["sumVc"], reps,
           tuple(int(x) for x in sched["Vc"]),
           tuple((b["nchunks"], len(b["pairs"])) if b else (0, 0)
                 for it in sched["iters"] if it
                 for b in (it["fm"], it["stale"])))
    if key not in _COMPILED:
        _COMPILED[key] = _build(sched, reps=reps)
    nc = _COMPILED[key]

    in_maps = []
    for c in range(NC):
        in_maps.append({
            "wmat": wmat, "vcols": vcols, "vrow": vrow,
            "n0row": sched["n0row"][c][None, :].astype(bf16),
            "degrow": sched["degrow"][c][None, :].astype(bf16),
            "idxs": sched["idxs"][c],
            "onehots": sched["onehots"][c],
        })

    res = run_bass_kernel_spmd(nc, in_maps, core_ids=list(range(NC)),
                               trace=trace)

    NL = sched["NL"]
    L_off, Vc, Voff = sched["L_off"], sched["Vc"], sched["Voff"]
    node_of_rank = sched["node_of_rank"]
    out = np.zeros(sched["N"], np.float32)
    for c in range(NC):
        oc = res.results[c]["pred"]
        for l in range(NL):
            gr = int(L_off[l]) + c * int(Vc[l]) + np.arange(int(Vc[l]))
            nd = node_of_rank[gr]
            m = nd >= 0
            out[nd[m]] = oc[int(Voff[l]):int(Voff[l]) + int(Vc[l])][m]
    return out[:, None], res


def kernel(**inputs):
    out, _ = _run(inputs, trace=False)
    return out


# revision 53
# speedup vs baseline: 1.1303x; 1.1303x over previous
"""DeepSAT GNN message-passing kernel for 8 Trainium2 NeuronCores.

Algorithm notes (validated numerically against the reference):
  - Every node is updated exactly once, at step l = forward_level (levels
    1..19; level-0 nodes keep h0 forever). At update time the node's own
    hidden state is still h0, so the GRU "hidden side" gates are constant
    vectors computable on the host.
  - msg_i = W @ (S_i + n0_i*h0) + deg_i*b, where S_i sums h over "active"
    in-edges (source level in [1, level_i)), n0_i counts inactive in-edges
    and deg_i all in-edges. With u = W^-1 b this folds to
    msg_i = W @ S'_i,  S'_i = S_i + n0_i*h0 + deg_i*u, so the per-gate
    input is  gi_g = (wih_g @ W) @ S'_i + bih_g  -- one fused matmul.
  - Nodes are stored level-sorted ("rank" order): per-level writes are
    contiguous, per-level ownership is an even 8-way split, and the
    AllGather of each level's new h lands in place.

Perf design (v2):
  - All matmuls/h-storage in bf16 (PE runs 1 cycle/row vs 4 for fp32);
    PSUM accumulation stays fp32.
  - Edge gathers use gpsimd.dma_gather (one SWDGE instruction per up to
    ~2-3k edges) instead of one indirect DMA per 128 edges: SWDGE fixed
    overhead is ~1us per instruction and was the previous bottleneck.
    dma_gather indices are int16, so "old" gathers (sources anywhere
    below the current level) are windowed into 32768-row slabs of
    h_store; "fresh" gathers (sources in level l-1 only) use the l-1
    slab directly.
  - Edges are sorted by destination slot so each 128-edge chunk spans
    few 128-slot blocks (~85% fill vs 44% before): fewer descriptors
    and fewer one-hot segment-sum matmuls.

Device schedule per level l (SPMD on 8 cores):
  dma_gather h[src] for this level's "fresh" edges (src level == l-1),
  segment-sum via one-hot matmuls into PSUM (seeded with the n0/deg
  terms), fused GRU, PE-transpose, DMA to the AllGather input, AllGather
  into the replicated h_store, then (overlapping the collective) the MLP
  head for this level plus the next level's "old" gathers/chunks, whose
  windows only read rows below this level's slab.
"""

import sys
import numpy as np

sys.path.insert(0, "/opt/trn_rl_repo")

P = 128
D = 128
NC = 8
GW = 512  # psum group width (one bank of fp32)
W32 = 32768  # dma_gather int16 index window

_COMPILED = {}


def _bf16():
    import ml_dtypes
    return ml_dtypes.bfloat16


# ---------------------------------------------------------------------------
# Host-side preprocessing
# ---------------------------------------------------------------------------

def _wrap_idx(vals):
    """int16 idx layout: j at [j%16, j//16], replicated to 8x16 partitions."""
    n = len(vals)
    cols = (n + 15) // 16
    t = np.zeros((P, cols), np.int16)
    for k in range(8):
        t[16 * k + (np.arange(n) % 16), np.arange(n) // 16] = vals
    return t


def _preprocess(forward_level, edge_index, num_levels):
    fl = np.asarray(forward_level).astype(np.int64)
    ei = np.asarray(edge_index).astype(np.int64)
    src, dst = ei[0], ei[1]
    N = fl.shape[0]
    NL = num_levels

    # --- rank space: nodes sorted by level, each level padded to NC*P ---
    n_l = np.bincount(fl, minlength=NL).astype(np.int64)
    pad_l = ((n_l + NC * P - 1) // (NC * P)) * (NC * P)
    pad_l = np.maximum(pad_l, NC * P)
    L_off = np.zeros(NL + 1, np.int64)
    L_off[1:] = np.cumsum(pad_l)
    Vc = (pad_l // NC).astype(np.int64)          # per-core nodes per level
    Voff = np.zeros(NL + 1, np.int64)
    Voff[1:] = np.cumsum(Vc)                     # per-core rank-space offsets
    nblk = (Vc // P).astype(np.int64)

    order = np.argsort(fl, kind="stable")
    starts_real = np.zeros(NL + 1, np.int64)
    starts_real[1:] = np.cumsum(n_l)
    pos_within = np.arange(N, dtype=np.int64) - starts_real[fl[order]]
    rank = np.empty(N, np.int64)
    rank[order] = L_off[fl[order]] + pos_within

    node_of_rank = np.full(L_off[NL], -1, np.int64)
    node_of_rank[rank] = np.arange(N, dtype=np.int64)

    # --- per-node degree stats, indexed by per-core rank space ---
    lv_s, lv_d = fl[src], fl[dst]
    act = (lv_s >= 1) & (lv_s < lv_d)
    deg = np.bincount(dst, minlength=N).astype(np.float64)
    n0 = np.bincount(dst[~act], minlength=N).astype(np.float64)

    sumVc = int(Voff[NL])
    n0row = np.zeros((NC, sumVc), np.float32)
    degrow = np.zeros((NC, sumVc), np.float32)
    for c in range(NC):
        grs = []
        for l in range(NL):
            grs.append(L_off[l] + c * Vc[l] + np.arange(Vc[l]))
        gr = np.concatenate(grs)
        nd = node_of_rank[gr]
        m = nd >= 0
        n0row[c, m] = n0[nd[m]]
        degrow[c, m] = deg[nd[m]]

    # --- active edge table ---
    er = np.where(act)[0]
    e_lvl = lv_d[er]
    e_srcrank = rank[src[er]].astype(np.int64)
    e_dstrank = rank[dst[er]].astype(np.int64)
    e_local = e_dstrank - L_off[e_lvl]
    e_core = e_local // Vc[e_lvl]
    e_wl = e_local % Vc[e_lvl]          # slot within core's level span
    e_fresh = lv_s[er] == (e_lvl - 1)

    # Per-iteration gather buckets; edges sorted by (dst level, dst slot) so
    # chunks span few 128-slot blocks. All counts are the max over cores so
    # the SPMD program is identical everywhere.
    #
    #   iter t "fm":    src level == t-1, dst in {t, t+1}. One window (the
    #                   l-1 slab); fires right after AllGather t-1 -- the
    #                   only gather on the critical path.
    #   iter t "stale": src level <= t-2, dst == t+1 (32k-row windows).
    #                   Emitted after collective_t: all its AllGather deps
    #                   completed long before, so descriptor generation and
    #                   the DMA overlap AllGather t entirely.
    idx_cols = []                # list of [NC, n] int64 idx blocks
    icol = 0
    rank_cols = [[] for _ in range(NC)]  # per pair: [128] f32 block-rel slots
    e_slot = e_wl                # dst slot within the core's level span

    def build_bucket(buckets):
        """buckets: list of (base, rows, sel_mask, padidx). Returns bucket
        dict with gather instrs, chunk count and pairs (lvl, ch, grp, big,
        pcol), or None if empty."""
        nonlocal icol
        out = {"instrs": [], "pairs": [], "nchunks": 0}
        nch_tot = 0
        for base, rows, sel, padidx in buckets:
            percore = []
            for c in range(NC):
                es = np.where(sel & (e_core == c))[0]
                es = es[np.lexsort((e_slot[es], e_lvl[es]))]
                percore.append(es)
            nmax = max(len(x) for x in percore)
            if nmax == 0:
                continue
            n = ((nmax + P - 1) // P) * P
            nch = n // P
            iv = np.zeros((NC, n), np.int64)
            for c in range(NC):
                es = percore[c]
                iv[c, :len(es)] = e_srcrank[es] - base
                iv[c, len(es):] = padidx
            assert iv.min() >= 0 and iv.max() < min(rows, W32)
            out["instrs"].append({"base": base, "rows": rows, "n": n,
                                  "icol": icol, "chunk0": nch_tot})
            idx_cols.append(iv)
            icol += n // 16
            for ch in range(nch):
                sl = slice(ch * P, (ch + 1) * P)
                lvl_blocks = set()
                for c in range(NC):
                    es = percore[c][sl]
                    lvl_blocks.update(
                        zip(e_lvl[es].tolist(), (e_slot[es] // P).tolist()))
                for (lv, b) in sorted(lvl_blocks):
                    pcol = len(rank_cols[0])
                    for c in range(NC):
                        es = percore[c][sl]
                        rv = np.full(P, -1.0, np.float32)
                        m = (e_lvl[es] == lv) & ((e_slot[es] // P) == b)
                        rv[:len(es)][m] = (e_slot[es][m] - b * P).astype(
                            np.float32)
                        rank_cols[c].append(rv)
                    out["pairs"].append(
                        (int(lv), nch_tot + ch, int(b) // 4, int(b) % 4, pcol))
            nch_tot += nch
        out["nchunks"] = nch_tot
        return out if nch_tot else None

    iters = [None] * NL
    for t in range(1, NL):
        fm = None
        if t >= 2:
            base = int(L_off[t - 1])
            assert pad_l[t - 1] <= W32, "slab exceeds int16 idx range"
            sel = (lv_s[er] == t - 1) & ((e_lvl == t) | (e_lvl == t + 1))
            fm = build_bucket([(base, int(pad_l[t - 1]), sel, 0)])
        stale = None
        if t + 1 < NL and t >= 3:
            sel0 = (e_lvl == t + 1) & (lv_s[er] <= t - 2) & (lv_s[er] >= 1)
            max_row = int(L_off[t - 1])
            nw = (max_row + W32 - 1) // W32
            bks = []
            for w in range(nw):
                wsel = sel0 & (e_srcrank >= w * W32) & (e_srcrank < (w + 1) * W32)
                rows = min(W32, int(L_off[NL]) - w * W32)
                padidx = int(L_off[1]) if w == 0 else 0
                bks.append((w * W32, rows, wsel, padidx))
            stale = build_bucket(bks)
        iters[t] = {"fm": fm, "stale": stale}

    # stop-flag bookkeeping: matmuls into S_ps[l] run in emission order
    # seeds_l -> stale pairs (iter l-1) -> mid pairs (fm of iter l-1,
    # lvl==l) -> fresh pairs (fm of iter l, lvl==l)
    levels = []
    for l in range(NL):
        info = {"ngrp": (int(Vc[l]) + GW - 1) // GW, "last": {}}
        if l >= 1:
            seqs = []
            if l >= 2 and iters[l - 1] and iters[l - 1]["stale"]:
                seqs.append(iters[l - 1]["stale"]["pairs"])
            if l >= 2 and iters[l - 1] and iters[l - 1]["fm"]:
                seqs.append(iters[l - 1]["fm"]["pairs"])
            if iters[l] and iters[l]["fm"]:
                seqs.append(iters[l]["fm"]["pairs"])
            for pairs in seqs:
                for (lv, ch, grp, big, pcol) in pairs:
                    if lv == l:
                        info["last"][grp] = pcol
        levels.append(info)

    ICOLS = max(icol, 1)
    idxs = np.zeros((NC, P, ICOLS), np.int16)
    bi = 0
    for t in range(1, NL):
        for bucket in (iters[t]["fm"], iters[t]["stale"]) if iters[t] else ():
            if bucket is None:
                continue
            for ins in bucket["instrs"]:
                iv = idx_cols[bi]
                bi += 1
                cols = iv.shape[1] // 16
                for c in range(NC):
                    idxs[c][:, ins["icol"]:ins["icol"] + cols] = _wrap_idx(iv[c])
    assert bi == len(idx_cols)
    # host-built one-hot matrices for the big "stale" buckets (DMA'd to
    # SBUF); the small "fm" one-hots are generated on the vector engine
    # from the rank columns.
    NPAIR = max(len(rank_cols[0]), 1)
    bf16 = _bf16()
    ranks = np.full((NC, P, NPAIR), -1.0, np.float32)
    onehots = np.zeros((NC, P, NPAIR * P), bf16)
    iota = np.arange(P, dtype=np.float32)
    for c in range(NC):
        if rank_cols[c]:
            rk = np.stack(rank_cols[c], axis=1)      # [128, npair]
            ranks[c, :, :rk.shape[1]] = rk
            oh = (rk[:, :, None] == iota[None, None, :])  # [128, npair, 128]
            onehots[c, :, :oh.shape[1] * P] = oh.reshape(P, -1).astype(bf16)

    return {
        "N": N, "NL": NL, "n_l": n_l, "pad": pad_l, "L_off": L_off,
        "Vc": Vc, "Voff": Voff, "nblk": nblk, "sumVc": sumVc,
        "ICOLS": ICOLS, "NPAIR": NPAIR,
        "levels": levels, "iters": iters, "idxs": idxs, "onehots": onehots,
        "ranks": ranks,
        "n0row": n0row, "degrow": degrow, "node_of_rank": node_of_rank,
    }


def _prep_weights(inp):
    f64 = np.float64
    W = inp["aggr_w"].astype(f64)
    b = inp["aggr_b"].astype(f64)
    h0 = (inp["emd_w"][:, 0] + inp["emd_b"]).astype(f64)
    wih = inp["gru_wih"].astype(f64)
    whh = inp["gru_whh"].astype(f64)
    bih = inp["gru_bih"].astype(f64)
    bhh = inp["gru_bhh"].astype(f64)
    u = np.linalg.solve(W, b)
    assert np.abs(W @ u - b).max() < 1e-6
    ghc = whh @ h0 + bhh
    hr_c, hz_c, hn_c = ghc[:D], ghc[D:2 * D], ghc[2 * D:]
    bih_r, bih_z, bih_n = bih[:D], bih[D:2 * D], bih[2 * D:]
    WgT = [(wih[g * D:(g + 1) * D] @ W).T for g in range(3)]

    W1 = inp["w1"].astype(f64)  # [256, 128]
    W2 = inp["w2"].astype(f64)  # [256, 256]
    w3 = inp["w3"].astype(f64)  # [1, 256]
    assert W1.shape[0] == 256

    bf16 = _bf16()
    blocks = [
        WgT[0], WgT[1], WgT[2], np.diag(hn_c),
        W1[0:128, :].T, W1[128:256, :].T,
        W2[0:128, 0:128].T, W2[0:128, 128:256].T,
        W2[128:256, 0:128].T, W2[128:256, 128:256].T,
        np.eye(128), np.tile(np.arange(128, dtype=f64)[None, :], (128, 1)),
        np.concatenate([w3[0, 0:128, None], w3[0, 128:256, None],
                        np.zeros((128, 126))], axis=1),
    ]
    wmat = np.concatenate(blocks, axis=1).astype(bf16)  # [128, 13*128] bf16

    vcols = np.stack([
        h0,                      # 0: h0 column
        bih_r + hr_c,            # 1: sigmoid bias for r
        -(bih_z + hz_c),         # 2: sigmoid bias for z' (scale = -1)
        bih_n,                   # 3: tanh bias for n
        inp["b1"].astype(f64)[0:128],    # 4
        inp["b1"].astype(f64)[128:256],  # 5
        inp["b2"].astype(f64)[0:128],    # 6
        inp["b2"].astype(f64)[128:256],  # 7
        np.full(128, inp["b3"].astype(f64)[0]),  # 8: b3 (row 0 used)
    ], axis=1).astype(np.float32)  # [128, 9] fp32 (activation biases + h0)

    vrow = np.zeros((1, 256), np.float32)
    vrow[0, :128] = h0.astype(np.float32)
    vrow[0, 128:] = u.astype(np.float32)
    return wmat, vcols, vrow.astype(bf16)


# ---------------------------------------------------------------------------
# Bass program
# ---------------------------------------------------------------------------

WM = {name: i for i, name in enumerate(
    ["WgT_r", "WgT_z", "WgT_n", "diag_hn", "W1Ta", "W1Tb",
     "W2_k0m0", "W2_k1m0", "W2_k0m1", "W2_k1m1", "ident", "iota", "w3c"])}
VC = {name: i for i, name in enumerate(
    ["h0", "bias_r", "nbias_z", "bias_n", "b1a", "b1b", "b2a", "b2b", "b3"])}




def _build(sched, reps=1):
    import concourse.bacc as bacc
    import concourse.tile as tile
    from concourse import bass, mybir, library_config

    f32 = mybir.dt.float32
    bf = mybir.dt.bfloat16
    i16 = mybir.dt.int16
    AF = mybir.ActivationFunctionType
    OP = mybir.AluOpType
    NL = sched["NL"]
    L_off = sched["L_off"]
    Vc = sched["Vc"]
    Voff = sched["Voff"]
    pad = sched["pad"]
    ICOLS = sched["ICOLS"]
    NPAIR = sched["NPAIR"]
    sumVc = sched["sumVc"]
    NpadTot = int(L_off[NL])
    RG = [list(range(NC))]

    nc = bacc.Bacc("TRN2", target_bir_lowering=False, debug=False,
                   enable_asserts=False, num_devices=NC)

    wmat_d = nc.dram_tensor("wmat", [P, P * len(WM)], bf, kind="ExternalInput")
    vcols_d = nc.dram_tensor("vcols", [P, len(VC)], f32, kind="ExternalInput")
    vrow_d = nc.dram_tensor("vrow", [1, 256], bf, kind="ExternalInput")
    n0_d = nc.dram_tensor("n0row", [1, sumVc], bf, kind="ExternalInput")
    deg_d = nc.dram_tensor("degrow", [1, sumVc], bf, kind="ExternalInput")
    idx_d = nc.dram_tensor("idxs", [P, ICOLS], i16, kind="ExternalInput")
    oh_d = nc.dram_tensor("onehots", [P, NPAIR * P], bf, kind="ExternalInput")
    rnk_d = nc.dram_tensor("ranks", [P, NPAIR], bf, kind="ExternalInput")
    pred_d = nc.dram_tensor("pred", [sumVc], f32, kind="ExternalOutput")
    h_store = nc.dram_tensor("h_store", [NpadTot, D], bf, kind="Internal",
                             addr_space="Shared")
    ag_in = [nc.dram_tensor(f"ag_in{i}", [int(Vc.max()), D], bf, kind="Internal")
             for i in range(2)]

    with tile.TileContext(nc) as tc:
        nc.gpsimd.load_library(library_config.mlp)
        cpool = tc.alloc_tile_pool(name="const", bufs=1)
        spool = tc.alloc_tile_pool(name="sbuf", bufs=2)
        gpool = tc.alloc_tile_pool(name="gath", bufs=2)
        hpool = tc.alloc_tile_pool(name="hnew", bufs=6)
        ppool = tc.alloc_tile_pool(name="psS", bufs=3, space="PSUM")
        qpool = tc.alloc_tile_pool(name="psG", bufs=3, space="PSUM")
        tpool = tc.alloc_tile_pool(name="psT", bufs=1, space="PSUM")
        rpool = tc.alloc_tile_pool(name="psP", bufs=1, space="PSUM")

        # ---- load constants ----
        wm = cpool.tile([P, P * len(WM)], bf, tag="wm")
        nc.sync.dma_start(out=wm[:], in_=wmat_d[:])
        vc = cpool.tile([P, len(VC)], f32, tag="vc")
        nc.sync.dma_start(out=vc[:], in_=vcols_d[:])
        vr = cpool.tile([1, 256], bf, tag="vr")
        nc.sync.dma_start(out=vr[:], in_=vrow_d[:])
        idxs = cpool.tile([P, ICOLS], i16, tag="idxs")
        nc.sync.dma_start(out=idxs[:], in_=idx_d[:])
        rnks = cpool.tile([P, NPAIR], bf, tag="rnks")
        nc.sync.dma_start(out=rnks[:], in_=rnk_d[:])

        def wmb(name):
            return wm[:, WM[name] * P:(WM[name] + 1) * P]

        def vcc(name):
            return vc[:, VC[name]:VC[name] + 1]

        h0b = cpool.tile([P, GW], bf, tag="h0b")  # h0 broadcast along free
        nc.vector.tensor_copy(out=h0b[:], in_=vcc("h0").to_broadcast([P, GW]))

        # ---- per-level state ----
        S_ps = [None] * NL         # list of psum tiles per level (by grp)
        last_ag = [None]           # most recent AllGather instruction

        def grp_widths(l):
            ws = []
            v = int(Vc[l])
            while v > 0:
                ws.append(min(GW, v))
                v -= GW
            return ws

        def emit_gathers(bucket, tag, pin):
            """dma_gather(s) for one bucket; pin = AllGather whose output
            the windows may read (DRAM regions aren't shadow-tracked)."""
            if bucket is None:
                return None
            hg = gpool.tile([P, bucket["nchunks"] * D], bf, tag=tag)
            for ins in bucket["instrs"]:
                n = ins["n"]
                c0 = ins["chunk0"]
                gi = nc.gpsimd.dma_gather(
                    out_ap=hg[:, c0 * D:(c0 + n // P) * D].rearrange(
                        "p (k d) -> p k d", d=D),
                    in_ap=h_store[ins["base"]:ins["base"] + ins["rows"], :],
                    idxs_ap=idxs[:, ins["icol"]:ins["icol"] + n // 16],
                    num_idxs=n,
                    num_idxs_reg=n,
                    elem_size=D,
                )
                if pin is not None:
                    tile.add_dep_helper(gi.ins, pin.ins, sync=True,
                                        reason="gather reads AllGather output")
            return hg

        def emit_onehots(bucket, tag, via):
            """One-hot block for a bucket: small fm blocks are generated on
            the vector engine; big stale blocks are DMA'd (host-built),
            split across the two HWDGE queues."""
            if bucket is None or not bucket["pairs"]:
                return None, None
            k = len(bucket["pairs"])
            p0 = bucket["pairs"][0][4]
            oh = spool.tile([P, k * P], bf, tag=tag)
            if via == "vec":
                CH = 4
                for s in range(0, k, CH):
                    m = min(CH, k - s)
                    nc.vector.tensor_tensor(
                        out=oh[:, s * P:(s + m) * P].rearrange(
                            "p (m f) -> p m f", m=m),
                        in0=rnks[:, p0 + s:p0 + s + m][:, :, None]
                            .to_broadcast([P, m, P]),
                        in1=wmb("iota")[:, None, :].to_broadcast([P, m, P]),
                        op=OP.is_equal,
                    )
            else:
                h = (k + 1) // 2 * P
                nc.scalar.dma_start(out=oh[:, :h],
                                    in_=oh_d[:, p0 * P:p0 * P + h])
                if k * P > h:
                    nc.sync.dma_start(out=oh[:, h:k * P],
                                      in_=oh_d[:, p0 * P + h:(p0 + k) * P])
            return oh, p0

        def emit_seeds(l):
            """allocate S psums for level l and seed with n0*h0 + deg*u."""
            tiles = []
            info = sched["levels"][l]
            v = int(Vc[l])
            off = int(Voff[l])
            n0r = spool.tile([1, int(Vc.max())], bf, tag="n0r")
            nc.sync.dma_start(out=n0r[0:1, :v], in_=n0_d[0:1, off:off + v])
            degr = spool.tile([1, int(Vc.max())], bf, tag="degr")
            nc.sync.dma_start(out=degr[0:1, :v], in_=deg_d[0:1, off:off + v])
            for g, w in enumerate(grp_widths(l)):
                sp = ppool.tile([P, GW], f32, tag="S", space="PSUM")
                nc.tensor.matmul(
                    out=sp[:, :w], lhsT=vr[0:1, 0:128],
                    rhs=n0r[0:1, g * GW:g * GW + w],
                    start=True, stop=False, skip_group_check=True)
                is_last = info["last"].get(g) is None
                nc.tensor.matmul(
                    out=sp[:, :w], lhsT=vr[0:1, 128:256],
                    rhs=degr[0:1, g * GW:g * GW + w],
                    start=False, stop=is_last, skip_group_check=True)
                tiles.append(sp)
            S_ps[l] = tiles

        def emit_chunks(bucket, lvl, hg, oh, p0):
            """segment-sum matmuls for the bucket's pairs targeting S_ps[lvl]."""
            if bucket is None:
                return
            info = sched["levels"][lvl]
            for (lv, ch, grp, big, pcol) in bucket["pairs"]:
                if lv != lvl:
                    continue
                is_last = info["last"].get(grp) == pcol
                nc.tensor.matmul(
                    out=S_ps[lvl][grp][:, big * P:(big + 1) * P],
                    lhsT=hg[:, ch * D:(ch + 1) * D],
                    rhs=oh[:, (pcol - p0) * P:(pcol - p0 + 1) * P],
                    start=False, stop=is_last, skip_group_check=True)

        def emit_mlp(l, g, w, rhs_sb, bcast=False):
            """MLP head for one 512-group; writes pred rows."""
            z1s = []
            for half in ("a", "b"):
                zp = qpool.tile([P, GW], f32, tag="G", space="PSUM")
                nc.tensor.matmul(out=zp[:, :w], lhsT=wmb("W1T" + half),
                                 rhs=rhs_sb[:, :w], start=True, stop=True)
                zs = spool.tile([P, GW], bf, tag="z1" + half)
                nc.scalar.activation(out=zs[:, :w], in_=zp[:, :w], func=AF.Relu,
                                     bias=vcc("b1" + half))
                z1s.append(zs)
            z2s = []
            for mi, mh in enumerate(("m0", "m1")):
                zp = qpool.tile([P, GW], f32, tag="G", space="PSUM")
                nc.tensor.matmul(out=zp[:, :w], lhsT=wmb("W2_k0" + mh),
                                 rhs=z1s[0][:, :w], start=True, stop=False)
                nc.tensor.matmul(out=zp[:, :w], lhsT=wmb("W2_k1" + mh),
                                 rhs=z1s[1][:, :w], start=False, stop=True)
                zs = spool.tile([P, GW], bf, tag="z2" + mh)
                nc.scalar.activation(out=zs[:, :w], in_=zp[:, :w], func=AF.Relu,
                                     bias=vcc("b2" + ("a" if mi == 0 else "b")))
                z2s.append(zs)
            pp = rpool.tile([1, GW], f32, tag="pred", space="PSUM")
            nc.tensor.matmul(out=pp[:, :w], lhsT=wmb("w3c")[:, 0:1],
                             rhs=z2s[0][:, :w], start=True, stop=False)
            nc.tensor.matmul(out=pp[:, :w], lhsT=wmb("w3c")[:, 1:2],
                             rhs=z2s[1][:, :w], start=False, stop=True)
            ps = spool.tile([1, GW], f32, tag="psb")
            nc.scalar.activation(out=ps[:, :w], in_=pp[:, :w], func=AF.Identity,
                                 bias=vc[0:1, VC["b3"]:VC["b3"] + 1])
            if bcast:
                pbc = spool.tile([1, GW], f32, tag="pbc")
                nc.vector.tensor_copy(out=pbc[0:1, :],
                                      in_=ps[0:1, 0:1].to_broadcast([1, GW]))
                for gg, ww in enumerate(grp_widths(l)):
                    off = int(Voff[l]) + gg * GW
                    nc.sync.dma_start(out=pred_d[off:off + ww],
                                      in_=pbc[0:1, :ww])
            else:
                off = int(Voff[l]) + g * GW
                nc.sync.dma_start(out=pred_d[off:off + w], in_=ps[0:1, :w])

        # reps>1 repeats the whole computation for wall-clock timing: the
        # computation is idempotent (h_store/pred rewritten with same values)
        for _rep in range(reps):
          # ================= level 0: one column, broadcast ==============
          # every level-0 node keeps h = h0, so pred is a single scalar
          emit_mlp(0, 0, 1, h0b, bcast=True)

          # seeds + (no chunks possible) for level 1
          emit_seeds(1)
          OH = {}  # (t, kind) -> (oh, p0); prefetched one iteration ahead

          def prefetch_oh(t):
              if t < NL and sched["iters"][t]:
                  OH[(t, "fm")] = emit_onehots(
                      sched["iters"][t]["fm"], "oh_fm", "vec")
                  OH[(t, "st")] = emit_onehots(
                      sched["iters"][t]["stale"], "oh_st", "dma")

          prefetch_oh(1)

          # ================= levels 1..NL-1 =================
          for l in range(1, NL):
            widths = grp_widths(l)
            it = sched["iters"][l]
            fm = it["fm"] if it else None
            stale = it["stale"] if it else None

            # fm gather: the only one on the critical path (needs AG l-1);
            # reads the l-1 slab for fresh_l + mid_{l+1} edges
            hg_fm = emit_gathers(fm, "hg_fm", last_ag[0])
            oh_fm, p0_fm = OH.get((l, "fm"), (None, None))
            if hg_fm is not None:
                emit_chunks(fm, l, hg_fm, oh_fm, p0_fm)

            # GRU per group
            hnew = []
            for g, w in enumerate(widths):
                ssb = spool.tile([P, GW], bf, tag="Ssb")
                nc.scalar.activation(out=ssb[:, :w], in_=S_ps[l][g][:, :w],
                                     func=AF.Identity)

                gr = qpool.tile([P, GW], f32, tag="G", space="PSUM")
                nc.tensor.matmul(out=gr[:, :w], lhsT=wmb("WgT_r"),
                                 rhs=ssb[:, :w], start=True, stop=True)
                gz = qpool.tile([P, GW], f32, tag="G", space="PSUM")
                nc.tensor.matmul(out=gz[:, :w], lhsT=wmb("WgT_z"),
                                 rhs=ssb[:, :w], start=True, stop=True)
                gn = qpool.tile([P, GW], f32, tag="G", space="PSUM")
                nc.tensor.matmul(out=gn[:, :w], lhsT=wmb("WgT_n"),
                                 rhs=ssb[:, :w], start=True, stop=False)

                rsb = spool.tile([P, GW], bf, tag="rsb")
                nc.scalar.activation(out=rsb[:, :w], in_=gr[:, :w],
                                     func=AF.Sigmoid, bias=vcc("bias_r"))
                zsb = spool.tile([P, GW], bf, tag="zsb")
                nc.scalar.activation(out=zsb[:, :w], in_=gz[:, :w],
                                     func=AF.Sigmoid, bias=vcc("nbias_z"),
                                     scale=-1.0)
                nc.tensor.matmul(out=gn[:, :w], lhsT=wmb("diag_hn"),
                                 rhs=rsb[:, :w], start=False, stop=True)
                nsb = spool.tile([P, GW], bf, tag="nsb")
                nc.scalar.activation(out=nsb[:, :w], in_=gn[:, :w],
                                     func=AF.Tanh, bias=vcc("bias_n"))

                t3 = spool.tile([P, GW], bf, tag="t3")
                nc.vector.tensor_scalar(out=t3[:, :w], in0=nsb[:, :w],
                                        scalar1=vcc("h0"), scalar2=None,
                                        op0=OP.subtract)
                t4 = spool.tile([P, GW], bf, tag="t4")
                nc.vector.tensor_tensor(out=t4[:, :w], in0=t3[:, :w],
                                        in1=zsb[:, :w], op=OP.mult)
                hn = hpool.tile([P, GW], bf, tag="hnew")
                nc.vector.tensor_scalar(out=hn[:, :w], in0=t4[:, :w],
                                        scalar1=vcc("h0"), scalar2=None,
                                        op0=OP.add)
                hnew.append(hn)

            # transpose h_new to node-major, stage straight from PSUM, and
            # AllGather into every core's h_store (skipped for the last
            # level: nothing reads it)
            if l < NL - 1:
                agt = ag_in[l % 2]
                for g, w in enumerate(widths):
                    tp = tpool.tile([P, GW], bf, tag="tp", space="PSUM")
                    nb = w // P
                    for b in range(nb):
                        nc.tensor.transpose(
                            out=tp[:, b * P:(b + 1) * P],
                            in_=hnew[g][:, b * P:(b + 1) * P],
                            identity=wmb("ident"))
                    tps = spool.tile([P, GW], bf, tag="tps")
                    nc.vector.tensor_copy(out=tps[:, :w], in_=tp[:, :w])  # noqa: tp is bf16 psum
                    for b in range(nb):
                        row = g * GW + b * P
                        nc.sync.dma_start(out=agt[row:row + P, :],
                                          in_=tps[:, b * P:(b + 1) * P])
            prev_ag = last_ag[0]
            if l < NL - 1:
                cc = nc.gpsimd.collective_compute(
                    "AllGather", mybir.AluOpType.bypass,
                    replica_groups=RG,
                    ins=[agt[0:int(Vc[l]), :].opt()],
                    outs=[h_store[int(L_off[l]):int(L_off[l]) + int(pad[l]), :].opt()],
                )
                last_ag[0] = cc

            # while AllGather l flies: seeds, stale gathers (their newest
            # source level is l-2, so AG l-1 -- long since waited on -- is
            # their pin), stale + mid chunks for level l+1, this level's
            # MLP head, and the one-hot prefetch for the next iteration
            if l + 1 < NL:
                emit_seeds(l + 1)
                hg_st = emit_gathers(stale, "hg_st", prev_ag)
                oh_st, p0_st = OH.get((l, "st"), (None, None))
                if hg_st is not None:
                    emit_chunks(stale, l + 1, hg_st, oh_st, p0_st)
                if hg_fm is not None:
                    emit_chunks(fm, l + 1, hg_fm, oh_fm, p0_fm)

            for g, w in enumerate(widths):
                emit_mlp(l, g, w, hnew[g])

            prefetch_oh(l + 1)

        for pl in (rpool, tpool, qpool, ppool, hpool, gpool, spool, cpool):
            pl.release()

    nc.compile()
    return nc


# ---------------------------------------------------------------------------
# Entry point
# ---------------------------------------------------------------------------

def _run(inputs, trace=False, reps=1):
    from concourse.bass_utils import run_bass_kernel_spmd

    bf16 = _bf16()
    fl = np.asarray(inputs["forward_level"])
    num_levels = int(fl.max()) + 1
    sched = _preprocess(fl, inputs["edge_index"], num_levels)
    wmat, vcols, vrow = _prep_weights(inputs)

    key = (sched["N"], sched["ICOLS"], sched["NPAIR"], sched["sumVc"], reps,
           tuple(int(x) for x in sched["Vc"]),
           tuple((b["nchunks"], len(b["pairs"])) if b else (0, 0)
                 for it in sched["iters"] if it
                 for b in (it["fm"], it["stale"])))
    if key not in _COMPILED:
        _COMPILED[key] = _build(sched, reps=reps)
    nc = _COMPILED[key]

    in_maps = []
    for c in range(NC):
        in_maps.append({
            "wmat": wmat, "vcols": vcols, "vrow": vrow,
            "n0row": sched["n0row"][c][None, :].astype(bf16),
            "degrow": sched["degrow"][c][None, :].astype(bf16),
            "idxs": sched["idxs"][c],
            "onehots": sched["onehots"][c],
            "ranks": sched["ranks"][c].astype(bf16),
        })

    res = run_bass_kernel_spmd(nc, in_maps, core_ids=list(range(NC)),
                               trace=trace)

    NL = sched["NL"]
    L_off, Vc, Voff = sched["L_off"], sched["Vc"], sched["Voff"]
    node_of_rank = sched["node_of_rank"]
    out = np.zeros(sched["N"], np.float32)
    for c in range(NC):
        oc = res.results[c]["pred"]
        for l in range(NL):
            gr = int(L_off[l]) + c * int(Vc[l]) + np.arange(int(Vc[l]))
            nd = node_of_rank[gr]
            m = nd >= 0
            out[nd[m]] = oc[int(Voff[l]):int(Voff[l]) + int(Vc[l])][m]
    return out[:, None], res


def kernel(**inputs):
    out, _ = _run(inputs, trace=False)
    return out
